# revision 17
# baseline (speedup 1.0000x reference)
"""Multi-head causal self-attention on 8 Trainium2 NeuronCores.

Problem: B=4, T=2048, D=1024, H=16 heads, Hd=64. fp32.
Sharding: core c handles batch b = c//2 and head-group g = c%2 (8 heads,
512 channels). Each core computes a partial output (its head-group's
contribution to x @ Wo); the host sums head-group pairs and adds bo.

Per-core algorithm (all layouts chosen so no on-chip transposes are
needed after the single x -> x^T transpose):
  x^T  [D=1024, T]   via PE transpose (128x128 blocks)
  Q^T  [C=512, T]    = matmul(lhsT=Wq chunk, rhs=x^T)   (head h at partitions
  K^T  [C=512, T]      64*(h%2) .. of chunk h//2)
  V'   [T, 8*65]     = matmul(lhsT=x^T chunk, rhs=Wv), per head [V(64) | 1]
  S^T  [k,q]         = matmul(lhsT=K^T block, rhs=Q^T span)  (k on partitions)
  E = exp((S^T+mask)/8)  on ScalarE, PSUM->SBUF
  ctx' [65, q]       = matmul(lhsT=V' block, rhs=E)  accumulated over k blocks
                       row 64 = softmax denominator (ones-column trick)
  ctx^T normalized via reciprocal + gpsimd partition_broadcast + DVE mult
  out  [T, D]        = matmul(lhsT=ctx^T chunk, rhs=Wo chunk), DMA out
Causality: only k-blocks with k0 <= q_span_end are computed; the <=4
diagonal blocks per span get an additive staircase mask.
"""

import sys

for _p in ("/opt/trn_rl_repo", "/root/.axon_site/_ro/trn_rl_repo"):
    if _p not in sys.path:
        sys.path.append(_p)

import numpy as np

import concourse.bacc as bacc
import concourse.mybir as mybir
import concourse.tile as tile
from concourse.bass_utils import run_bass_kernel_spmd

FP32 = mybir.dt.float32
BF16 = mybir.dt.bfloat16
P = 128
T = 2048  # sequence length
D = 1024  # model dim
C = 512   # channels per core (8 heads)
H = 8     # heads per core
HD = 64   # head dim
NEG = -1e30
N_CORES = 8
NSPAN = 4          # q spans of 512
SPAN = 512
NKB = 16           # k blocks of 128

_program = None


def _build(debug=False):
    nc = bacc.Bacc()
    x_d = nc.declare_dram_parameter("x", [T, D], BF16, isOutput=False)
    wq_d = nc.declare_dram_parameter("wq", [D, C], BF16, isOutput=False)
    wk_d = nc.declare_dram_parameter("wk", [D, C], BF16, isOutput=False)
    wv_d = nc.declare_dram_parameter("wv", [D, C], BF16, isOutput=False)
    wo_d = nc.declare_dram_parameter("wo", [C, D], BF16, isOutput=False)
    mask_d = nc.declare_dram_parameter("mask", [P, 1024], BF16, isOutput=False)
    out_d = nc.declare_dram_parameter("out", [T, D], FP32, isOutput=True)
    if debug:
        xt_dump = nc.declare_dram_parameter("xt_dump", [8 * P, T], BF16, isOutput=True)
        qt_dump = nc.declare_dram_parameter("qt_dump", [C, T], BF16, isOutput=True)
        kt_dump = nc.declare_dram_parameter("kt_dump", [C, T], BF16, isOutput=True)
        vp_dump = nc.declare_dram_parameter("vp_dump", [T, H * 65], BF16, isOutput=True)
        ct_dump = nc.declare_dram_parameter("ct_dump", [C, T], BF16, isOutput=True)

    Exp = mybir.ActivationFunctionType.Exp
    Add = mybir.AluOpType.add

    def copy_px(idx, dst, src):
        # alternate PSUM->SBUF copies between ScalarE and VectorE
        if idx % 2 == 0:
            nc.scalar.copy(dst, src)
        else:
            nc.vector.tensor_copy(dst, src)

    from contextlib import ExitStack

    with tile.TileContext(nc) as tc, ExitStack() as persist:
        const_pool = persist.enter_context(tc.tile_pool(name="const", bufs=1))
        qkt_pool = persist.enter_context(tc.tile_pool(name="qkt", bufs=1))
        vp_pool = persist.enter_context(tc.tile_pool(name="vp", bufs=1))
        persist_w = persist.enter_context(tc.tile_pool(name="pw", bufs=1))
        ctxT_pool = persist.enter_context(tc.tile_pool(name="ctxT", bufs=1))

        mask_sb = const_pool.tile([P, 1024], BF16, tag="mask")
        nc.sync.dma_start(mask_sb[:], mask_d[:])
        qt = [qkt_pool.tile([P, T], BF16, tag=f"qt{i}", name=f"qt{i}") for i in range(4)]
        kt = [qkt_pool.tile([P, T], BF16, tag=f"kt{i}", name=f"kt{i}") for i in range(4)]
        vp = [vp_pool.tile([P, H * 65], BF16, tag=f"vp{t}", name=f"vp{t}") for t in range(NKB)]

        with tc.tile_pool(name="xt", bufs=1) as xt_pool:
            xt = [xt_pool.tile([P, T], BF16, tag=f"xt{j}", name=f"xt{j}") for j in range(8)]

            # ---- Phase A: x -> x^T via XBAR DMA transpose (bf16) -------
            # th-outer so the first half of every chunk lands early and
            # phase B2 (V', needs t-chunks in order) can start sooner.
            for th in range(2):
                for j in range(8):
                    tsl = slice(th * (T // 2), (th + 1) * (T // 2))
                    nc.sync.dma_start_transpose(
                        xt[j][:, tsl], x_d[tsl, j * P:(j + 1) * P])

            # ---- Phase B2 first: V' (ones column at 64 of each head) ---
            with tc.tile_pool(name="wv", bufs=1) as wv_pool:
                wv_sb = wv_pool.tile([P, 8, C], BF16, tag="wv")
                nc.sync.dma_start(wv_sb[:], wv_d.rearrange("(o p) c -> p o c", p=P))
                with tc.tile_pool(name="vps", bufs=4, space="PSUM") as v_psum:
                    for t in range(NKB):
                        nc.gpsimd.memset(vp[t][:], 1.0)
                        ps = v_psum.tile([P, C], FP32, tag="vps")
                        for j in range(8):
                            nc.tensor.matmul(
                                ps[:],
                                xt[j][:, t * P:(t + 1) * P],
                                wv_sb[:, j, :],
                                start=(j == 0), stop=(j == 7),
                            )
                        dst = vp[t].rearrange("p (h e) -> p h e", e=65)[:, :, 0:64]
                        src2 = ps.rearrange("p (h e) -> p h e", e=64)
                        copy_px(t, dst, src2)

            if debug:
                for j in range(8):
                    nc.sync.dma_start(xt_dump[j * P:(j + 1) * P, :], xt[j][:])
                for t in range(NKB):
                    nc.sync.dma_start(vp_dump[t * P:(t + 1) * P, :], vp[t][:])

            # ---- B1 + attention, interleaved per head-pair -------------
            # Projection matmuls for head-pair hp+1 give the PE work to do
            # while ScalarE (exp, the attention bottleneck) drains hp.
            wq_sb = persist_w.tile([P, 8, C], BF16, tag="wq")
            nc.sync.dma_start(wq_sb[:], wq_d.rearrange("(o p) c -> p o c", p=P))
            wk_sb = persist_w.tile([P, 8, C], BF16, tag="wk")
            nc.sync.dma_start(wk_sb[:], wk_d.rearrange("(o p) c -> p o c", p=P))

            ctxT = [ctxT_pool.tile([P, T], BF16, tag=f"ct{i}", name=f"ct{i}")
                    for i in range(4)]
            wo_sb = persist_w.tile([P, 4, D], BF16, tag="wo")
            nc.sync.dma_start(wo_sb[:], wo_d.rearrange("(o p) d -> p o d", p=P))

            with (
                tc.tile_pool(name="qkps", bufs=2, space="PSUM") as qk_psum,
                tc.tile_pool(name="stps", bufs=2, space="PSUM") as st_pool,
                tc.tile_pool(name="csA", bufs=1, space="PSUM") as csA_pool,
                tc.tile_pool(name="csB", bufs=1, space="PSUM") as csB_pool,
                tc.tile_pool(name="epool", bufs=4) as e_pool,
                tc.tile_pool(name="npool", bufs=2) as n_pool,
                tc.tile_pool(name="rdram", bufs=2, space="DRAM") as rdram_pool,
            ):
                ci = 0
                for hp in range(4):
                    # B1 part: Q^T and K^T chunks for this head pair
                    for dst, wsb in ((qt, wq_sb), (kt, wk_sb)):
                        for s in range(NSPAN):
                            ps = qk_psum.tile([P, SPAN], FP32, tag="qkps")
                            for j in range(8):
                                nc.tensor.matmul(
                                    ps[:],
                                    wsb[:, j, hp * P:(hp + 1) * P],
                                    xt[j][:, s * SPAN:(s + 1) * SPAN],
                                    start=(j == 0), stop=(j == 7),
                                )
                            copy_px(ci, dst[hp][:, s * SPAN:(s + 1) * SPAN], ps[:])
                            ci += 1
                    # attention for this head pair
                    hA, hB = 2 * hp, 2 * hp + 1
                    for s in range(NSPAN):
                        qsl = slice(s * SPAN, (s + 1) * SPAN)
                        csA = csA_pool.tile([P, SPAN], FP32, tag="csA")
                        csB = csB_pool.tile([P, SPAN], FP32, tag="csB")
                        nkb = 4 * s + 4
                        for kb in range(nkb):
                            ksl = slice(kb * P, (kb + 1) * P)
                            st = st_pool.tile([P, 1024], FP32, tag="st")
                            nc.tensor.matmul(st[:, 0:512], kt[hp][0:64, ksl],
                                             qt[hp][0:64, qsl],
                                             start=True, stop=True)
                            nc.tensor.matmul(st[:, 512:1024], kt[hp][64:128, ksl],
                                             qt[hp][64:128, qsl],
                                             start=True, stop=True)
                            e = e_pool.tile([P, 1024], BF16, tag="e")
                            nc.scalar.activation(e[:], st[:], Exp, scale=0.125)
                            if kb >= 4 * s:
                                off = (4 * s - kb) * P + 384
                                m3 = mask_sb[:, None, off:off + 512]
                                e3 = e.rearrange("p (b q) -> p b q", b=2)
                                nc.vector.tensor_mul(
                                    e3, e3, m3.to_broadcast((P, 2, 512)))
                            nc.tensor.matmul(csA[0:65, :],
                                             vp[kb][:, hA * 65:(hA + 1) * 65],
                                             e[:, 0:512],
                                             start=(kb == 0), stop=(kb == nkb - 1))
                            nc.tensor.matmul(csB[0:65, :],
                                             vp[kb][:, hB * 65:(hB + 1) * 65],
                                             e[:, 512:1024],
                                             start=(kb == 0), stop=(kb == nkb - 1))
                        # normalize: rows 0..63 / row 64 (rowsum via the ones
                        # column).  reciprocal_approx_fast is broken at nonzero
                        # base partition, so broadcast first (DRAM bounce),
                        # then recip at base 0.
                        rs = n_pool.tile([P, 1024], FP32, tag="rs")
                        rsA = n_pool.tile([P, SPAN], FP32, tag="rsA")
                        rsB = n_pool.tile([P, SPAN], FP32, tag="rsB")
                        rrA = n_pool.tile([P, SPAN], FP32, tag="rrA")
                        rrB = n_pool.tile([P, SPAN], FP32, tag="rrB")
                        tmpB = n_pool.tile([P, SPAN], BF16, tag="tmpB")
                        nc.scalar.copy(rs[64:65, 0:512], csA[64:65, :])
                        nc.scalar.copy(rs[64:65, 512:1024], csB[64:65, :])
                        rd = rdram_pool.tile([1024], FP32, tag="rd")
                        nc.sync.dma_start(rd[None, :], rs[64:65, :])
                        nc.sync.dma_start(
                            rsA[0:64, :], rd[None, 0:512].to_broadcast((64, 512)))
                        nc.sync.dma_start(
                            rsB[0:64, :], rd[None, 512:1024].to_broadcast((64, 512)))
                        nc.vector.reciprocal_approx_fast(rrA[0:64, :], rsA[0:64, :])
                        nc.vector.reciprocal_approx_fast(rrB[0:64, :], rsB[0:64, :])
                        nc.vector.tensor_mul(ctxT[hp][0:64, qsl],
                                             csA[0:64, :], rrA[0:64, :])
                        nc.vector.tensor_mul(tmpB[0:64, :],
                                             csB[0:64, :], rrB[0:64, :])
                        nc.sync.dma_start(ctxT[hp][64:128, qsl], tmpB[0:64, :])

        # xt freed here; qt/kt/vp/mask persist
        if debug:
            for i in range(4):
                nc.sync.dma_start(qt_dump[i * P:(i + 1) * P, :], qt[i][:])
                nc.sync.dma_start(kt_dump[i * P:(i + 1) * P, :], kt[i][:])
                nc.sync.dma_start(ct_dump[i * P:(i + 1) * P, :], ctxT[i][:])
        if True:
            # ---- Phase D: output projection ------------------------
            with (
                tc.tile_pool(name="ops", bufs=4, space="PSUM") as o_psum,
                tc.tile_pool(name="osb", bufs=4) as o_pool,
            ):
                for qb in range(16):
                    pss = [o_psum.tile([P, SPAN], FP32, tag=f"ops{nh}", name=f"ops{nh}")
                           for nh in range(2)]
                    for hp in range(4):
                        for nh in range(2):
                            nc.tensor.matmul(
                                pss[nh][:],
                                ctxT[hp][:, qb * P:(qb + 1) * P],
                                wo_sb[:, hp, nh * SPAN:(nh + 1) * SPAN],
                                start=(hp == 0), stop=(hp == 3),
                            )
                    for nh in range(2):
                        ot = o_pool.tile([P, SPAN], FP32, tag="osb")
                        copy_px(qb * 2 + nh, ot[:], pss[nh][:])
                        nc.sync.dma_start(
                            out_d[qb * P:(qb + 1) * P, nh * SPAN:(nh + 1) * SPAN],
                            ot[:])

    nc.compile()
    return nc


def _get_program():
    global _program
    if _program is None:
        _program = _build()
    return _program


def _make_mask():
    import ml_dtypes
    j = np.arange(1024)[None, :]
    k = np.arange(P)[:, None]
    return np.where(j >= k + 384, 1.0, 0.0).astype(ml_dtypes.bfloat16)


def _make_in_maps(x, Wq, Wk, Wv, Wo):
    import ml_dtypes
    bf16 = ml_dtypes.bfloat16
    mask = _make_mask()
    in_maps = []
    for c in range(N_CORES):
        b, g = c // 2, c % 2
        cols = slice(g * C, (g + 1) * C)
        in_maps.append({
            "x": np.ascontiguousarray(np.asarray(x[b], np.float32).astype(bf16)),
            "wq": np.ascontiguousarray(np.asarray(Wq[:, cols], np.float32).astype(bf16)),
            "wk": np.ascontiguousarray(np.asarray(Wk[:, cols], np.float32).astype(bf16)),
            "wv": np.ascontiguousarray(np.asarray(Wv[:, cols], np.float32).astype(bf16)),
            "wo": np.ascontiguousarray(np.asarray(Wo[cols, :], np.float32).astype(bf16)),
            "mask": mask,
        })
    return in_maps


def _combine(results, bo, B):
    out = np.empty((B, T, D), dtype=np.float32)
    bo = np.asarray(bo, dtype=np.float32)
    for b in range(B):
        out[b] = results[2 * b]["out"] + results[2 * b + 1]["out"] + bo
    return out


def kernel(x, Wq, Wk, Wv, Wo, bo):
    x = np.asarray(x)
    nc = _get_program()
    in_maps = _make_in_maps(x, Wq, Wk, Wv, Wo)
    res = run_bass_kernel_spmd(nc, in_maps, core_ids=list(range(N_CORES)))
    return _combine(res.results, bo, x.shape[0])


def kernel_traced(x, Wq, Wk, Wv, Wo, bo):
    """Like kernel() but also returns the BassKernelResults (with
    exec_time_ns when NTFF tracing is available)."""
    x = np.asarray(x)
    nc = _get_program()
    in_maps = _make_in_maps(x, Wq, Wk, Wv, Wo)
    res = run_bass_kernel_spmd(nc, in_maps, core_ids=list(range(N_CORES)),
                               trace=True)
    return _combine(res.results, bo, x.shape[0]), res
